# revision 34
# baseline (speedup 1.0000x reference)
"""Trainium2 Bass kernel for nn_CropConvLSTM.

Model: ConvLSTM (Conv1d(1+H -> 4H, k=3, pad=1), S=12 steps) over x (B=256,
S=12, L=128), then head Linear(98304->768)+BN+ReLU, Linear(768->12)+BN+ReLU,
Linear(12->10).

Distribution over 8 NeuronCores, three launches:
  Stage 1: ConvLSTM, data-parallel over batch (32 samples/core). Conv done as
    3 shifted fp32r matmuls (K=66: 64 h-channels + x-row + ones-row for the
    conv bias) accumulating in PSUM; gates on ACT/DVE/GPSIMD with all tensors
    at legal partition bases (tanh computed as 2*sigmoid(2x)-1 so a single
    per-partition-scaled sigmoid covers the [o;g] psum tile).
  Stage 2: y1 = flat @ (w1*bn1_scale).T, sharded over the 98304 contract dim
    (12288 features/core); each core emits a partial (768, 256), host reduces.
  Stage 3: bias+ReLU, Linear2+BN+ReLU, Linear3 (+b3 via ones-row trick),
    data-parallel over batch again.

BN (eval mode) is folded into the weights/biases on the host.
"""
import os
import sys

sys.path.insert(0, "/opt/trn_rl_repo")

from functools import lru_cache

import numpy as np

import concourse.bass as bass
import concourse.tile as tile
from concourse import bacc, mybir
from concourse.bass_utils import run_bass_kernel_spmd

F32 = mybir.dt.float32
F32R = mybir.dt.float32r
BF16 = mybir.dt.bfloat16
F16 = mybir.dt.float16
AF = mybir.ActivationFunctionType

B, S, L, H, C = 256, 12, 128, 64, 10
NC = 8
BLOC = B // NC            # 32 samples per core in stages 1/3
KTOT = S * H * L          # 98304
KSH = KTOT // NC          # 12288 contract features per core in stage 2
KCH = KSH // 128          # 96 k-chunks per core
EPS = 1e-5
CORE_IDS = list(range(NC))


# ---------------------------------------------------------------- stage 1
@lru_cache(maxsize=1)
def _build_stage1():
    """ConvLSTM step via two tap-packed matmul passes per gate pair.

    For output position l the conv contracts h at l-1, l, l+1 plus x at the
    same three offsets plus the bias.  Pass A (K=128) contracts h-taps 1 and 2
    from combA, whose partition rows hold h twice at different column shifts:
      combA[0:64,  :, c] = h[c]      (tap 1 at slice offset 0)
      combA[64:128,:, c] = h[c+1]    (tap 2; col L-1 stays 0)
    Pass B (K=68) contracts h-tap 0, the three x taps, and the bias from
    combB:
      combB[0:64, :, c] = h[c-1]     (col 0 stays 0)
      combB[64:67,:, c] = x[c-1], x[c], x[c+1]   (host-preshifted, per step)
      combB[67,  :, c] = 1.0                     (bias row)
    Gate math per group of 8 samples on [c; tanh(g)]-packed tiles so the DVE
    ops run 128 partitions wide; tanh(c) is computed directly on ACT (tanh
    and sigmoid share the act table, so no table reloads).  Groups are
    software-pipelined with a one-group lag so no engine queue blocks on the
    recurrence chain of the group just computed.
    """
    nc = bacc.Bacc("TRN2", target_bir_lowering=False, debug=False, num_devices=NC)
    xp4 = nc.dram_tensor("xp4", [S, 4, BLOC, L], BF16, kind="ExternalInput").ap()
    wafi = nc.dram_tensor("wafi", [128, 128], BF16, kind="ExternalInput").ap()
    waog = nc.dram_tensor("waog", [128, 128], BF16, kind="ExternalInput").ap()
    wbfi = nc.dram_tensor("wbfi", [68, 128], BF16, kind="ExternalInput").ap()
    wbog = nc.dram_tensor("wbog", [68, 128], BF16, kind="ExternalInput").ap()
    hs = nc.dram_tensor("hs", [S, H, BLOC, L], BF16, kind="ExternalOutput").ap()

    G = 4                 # independent sample groups (recurrence pipelines)
    GB = BLOC // G        # 8 samples per group

    with tile.TileContext(nc) as tc:
        with (
            tc.tile_pool(name="persist", bufs=1) as pp,
            tc.tile_pool(name="work", bufs=4) as wk,
            tc.tile_pool(name="psfi", bufs=2, space="PSUM") as ps_fi,
            tc.tile_pool(name="psog", bufs=2, space="PSUM") as ps_og,
        ):
            combA = pp.tile([128, BLOC, L], BF16)
            combB = pp.tile([68, BLOC, L + 2], BF16)
            wA_fi = pp.tile([128, 128], BF16)
            wA_og = pp.tile([128, 128], BF16)
            wB_fi = pp.tile([68, 128], BF16)
            wB_og = pp.tile([68, 128], BF16)
            scv = pp.tile([128, 1], F32)                # act scale [1;2]
            # per-group persistent cell-state tiles
            ct = [pp.tile([64, GB, L], BF16, name=f"ct{i}") for i in range(G)]

            # h and c start at zero, and step 0 skips every term that reads
            # them, so only the two permanent zero-pad columns need memsets:
            # combA col L-1 of the tap-2 rows and combB col 0 of the h rows
            nc.vector.memset(combA[64:128, :, L - 1 : L], 0.0)
            nc.vector.memset(combB[0:64, :, 0:1], 0.0)
            nc.vector.memset(scv[0:64], 1.0)
            nc.vector.memset(scv[64:128], 2.0)
            nc.sync.dma_start(out=wA_fi, in_=wafi)
            nc.sync.dma_start(out=wA_og, in_=waog)
            nc.sync.dma_start(out=wB_fi, in_=wbfi)
            nc.sync.dma_start(out=wB_og, in_=wbog)

            pend = []

            def emit_tail(s, g, g0, ctg, so):
                # tanh(c) then h = sig(o)*tanh(c); h lands in combA rows 0:64
                # (contiguous); the tap-2 shifted copy and the tap-0 copy into
                # combB ride SBUF->SBUF DMAs, spread across the SP and Pool
                # rings so neither ring backlogs the recurrence chain.
                tcf = wk.tile([64, GB, L], BF16, name="tcf")
                nc.scalar.activation(tcf, ctg, AF.Tanh)
                nc.vector.tensor_mul(combA[0:64, g0 : g0 + GB, :], so[0:64], tcf)
                nc.sync.dma_start(
                    out=combA[64:128, g0 : g0 + GB, 0 : L - 1],
                    in_=combA[0:64, g0 : g0 + GB, 1:L],
                )
                nc.gpsimd.dma_start(
                    out=combB[0:64, g0 : g0 + GB, 1 : L + 1],
                    in_=combA[0:64, g0 : g0 + GB, :],
                )
                nc.gpsimd.dma_start(
                    out=hs[s, :, g0 : g0 + GB, :],
                    in_=combA[0:64, g0 : g0 + GB, :],
                )

            for g in range(G):
                # step-0 x taps (later steps prefetch a full step ahead)
                nc.sync.dma_start(
                    out=combB[64:68, g * GB : (g + 1) * GB, 0:L],
                    in_=xp4[0, :, g * GB : (g + 1) * GB, :],
                )
            for s in range(S):
                for g in range(G):
                    g0 = g * GB
                    pfi = ps_fi.tile([128, GB, L], F32, name="pfi")
                    pog = ps_og.tile([128, GB, L], F32, name="pog")
                    if s > 0:
                        # h == 0 at step 0: pass A contributes nothing then
                        for wt, ps in ((wA_fi, pfi), (wA_og, pog)):
                            for c4 in range(GB // 4):
                                c0 = c4 * 4
                                nc.tensor.matmul(
                                    ps[:, c0 : c0 + 4, :], lhsT=wt,
                                    rhs=combA[:, g0 + c0 : g0 + c0 + 4, :],
                                    start=True, stop=False,
                                )
                    for wt, ps in ((wB_fi, pfi), (wB_og, pog)):
                        for c4 in range(GB // 4):
                            c0 = c4 * 4
                            if s == 0:
                                # contract only the x/ones rows (K=4)
                                nc.tensor.matmul(
                                    ps[:, c0 : c0 + 4, :], lhsT=wt[64:68],
                                    rhs=combB[64:68, g0 + c0 : g0 + c0 + 4, 0:L],
                                    start=True, stop=True,
                                )
                            else:
                                nc.tensor.matmul(
                                    ps[:, c0 : c0 + 4, :], lhsT=wt,
                                    rhs=combB[0:68, g0 + c0 : g0 + c0 + 4, 0:L],
                                    start=False, stop=True,
                                )
                    if s + 1 < S:
                        # prefetch next step's x taps now that pass B is done
                        # reading this step's rows (ACT ring; tiny transfer)
                        nc.sync.dma_start(
                            out=combB[64:68, g0 : g0 + GB, 0:L],
                            in_=xp4[s + 1, :, g0 : g0 + GB, :],
                        )
                    sg = wk.tile([128, GB, L], BF16, name="sg")
                    # [sig(o); sig(2g)] in fp16: enough mantissa that the
                    # 2x-1 tanh unfold stays accurate, and 2-byte operands
                    # keep the downstream DVE ops in the fast 2x mode
                    so = wk.tile([128, GB, L], F16, name="so")
                    nc.scalar.activation(sg, pfi, AF.Sigmoid)
                    nc.scalar.activation(so, pog, AF.Sigmoid, scale=scv)
                    ctg = ct[g]
                    # tanh(g) = 2*sig(2g)-1, staged at partitions 64:127 so
                    # the t2 multiply sees both inputs on the same partitions
                    tg = wk.tile([128, GB, L], BF16, name="tg")
                    nc.vector.tensor_scalar(
                        out=tg[64:128], in0=so[64:128], scalar1=2.0,
                        scalar2=-1.0, op0=mybir.AluOpType.mult,
                        op1=mybir.AluOpType.add,
                    )
                    if s == 0:
                        # c0 = sig(i)*tanh(g) directly (f-term is zero)
                        nc.vector.tensor_mul(ctg, sg[64:128], tg[64:128])
                    else:
                        t1 = wk.tile([64, GB, L], BF16, name="t1")
                        nc.vector.tensor_mul(t1, sg[0:64], ctg)
                        t2 = wk.tile([64, GB, L], BF16, name="t2")
                        nc.vector.tensor_mul(t2, sg[64:128], tg[64:128])
                        nc.vector.tensor_add(ctg, t1, t2)
                    pend.append((s, g, g0, ctg, so))
                    if len(pend) > 1:
                        emit_tail(*pend.pop(0))
            while pend:
                emit_tail(*pend.pop(0))
    nc.compile()
    return nc


# ---------------------------------------------------------------- stage 2
@lru_cache(maxsize=1)
def _build_stage2():
    """Contract-sharded GEMM.  Weights and activations are pre-transposed on
    the host to partition-major layout so every DMA reads fully contiguous
    bytes per partition; the weight stream (3x the activation bytes) is
    split across two hardware DMA queues (SP + ACT)."""
    nc = bacc.Bacc("TRN2", target_bir_lowering=False, debug=False, num_devices=NC)
    w1p = nc.dram_tensor("w1p", [128, KCH * 768], BF16, kind="ExternalInput").ap()
    ft = nc.dram_tensor("ft", [128, KCH * B], BF16, kind="ExternalInput").ap()
    y1p = nc.dram_tensor("y1p", [768, B], F32, kind="ExternalOutput").ap()

    KB = 8                       # k-chunks per DMA batch
    NB = KCH // KB               # 24 batches

    with tile.TileContext(nc) as tc:
        with (
            tc.tile_pool(name="wp", bufs=3) as wp,
            tc.tile_pool(name="rp", bufs=3) as rp,
            tc.tile_pool(name="op", bufs=2) as op,
            tc.tile_pool(name="ps", bufs=1, space="PSUM") as ps,
        ):
            acc = [ps.tile([128, B], F32, name=f"acc{m}") for m in range(6)]
            WB = KB * 768
            RB = KB * B
            for kb in range(NB):
                wt = wp.tile([128, WB], BF16, name="wt")
                rt = rp.tile([128, RB], BF16, name="rt")
                nc.sync.dma_start(
                    out=wt[:, 0 : WB // 2],
                    in_=w1p[:, kb * WB : kb * WB + WB // 2],
                )
                nc.scalar.dma_start(
                    out=wt[:, WB // 2 : WB],
                    in_=w1p[:, kb * WB + WB // 2 : (kb + 1) * WB],
                )
                nc.gpsimd.dma_start(
                    out=rt, in_=ft[:, kb * RB : (kb + 1) * RB],
                )
                for kc in range(KB):
                    for m in range(6):
                        nc.tensor.matmul(
                            acc[m],
                            lhsT=wt[:, kc * 768 + m * 128 : kc * 768 + (m + 1) * 128],
                            rhs=rt[:, kc * B : (kc + 1) * B],
                            start=(kb == 0 and kc == 0),
                            stop=(kb == NB - 1 and kc == KB - 1),
                        )
            for m in range(6):
                ot = op.tile([128, B], F32, name="ot")
                nc.vector.tensor_copy(ot, acc[m])
                nc.sync.dma_start(out=y1p[m * 128 : (m + 1) * 128], in_=ot)
    nc.compile()
    return nc


# ---------------------------------------------------------------- stage 3
@lru_cache(maxsize=1)
def _build_stage3():
    nc = bacc.Bacc("TRN2", target_bir_lowering=False, debug=False, num_devices=NC)
    y1s = nc.dram_tensor("y1s", [6, 128, BLOC], F32R, kind="ExternalInput").ap()
    c1t = nc.dram_tensor("c1t", [128, 6], F32, kind="ExternalInput").ap()
    w2p = nc.dram_tensor("w2p", [6, 128, 12], F32R, kind="ExternalInput").ap()
    c2t = nc.dram_tensor("c2t", [12, 1], F32, kind="ExternalInput").ap()
    w3e = nc.dram_tensor("w3e", [13, 10], F32R, kind="ExternalInput").ap()
    y3p = nc.dram_tensor("y3p", [BLOC, C], F32, kind="ExternalOutput").ap()

    with tile.TileContext(nc) as tc:
        with (
            tc.tile_pool(name="sb", bufs=1) as sb,
            tc.tile_pool(name="ps", bufs=1, space="PSUM") as ps,
        ):
            # warm the ACT table (relu set) while the input DMAs run
            warm = sb.tile([1, 1], F32)
            nc.vector.memset(warm, 0.0)
            nc.scalar.activation(warm, warm, AF.Relu)
            yt = sb.tile([128, 6, BLOC], F32R)
            c1 = sb.tile([128, 6], F32)
            w2t = sb.tile([128, 6, 12], F32R)
            c2 = sb.tile([12, 1], F32)
            w3t = sb.tile([13, 10], F32R)
            nc.sync.dma_start(out=yt, in_=y1s.rearrange("k p b -> p k b"))
            nc.gpsimd.dma_start(out=c1, in_=c1t)
            nc.scalar.dma_start(out=w2t, in_=w2p.rearrange("k p m -> p k m"))
            nc.gpsimd.dma_start(out=c2, in_=c2t)
            nc.scalar.dma_start(out=w3t, in_=w3e)

            r1 = sb.tile([128, 6, BLOC], F32R)
            for kc in range(6):
                nc.scalar.activation(
                    r1[:, kc, :], yt[:, kc, :], AF.Relu, bias=c1[:, kc : kc + 1]
                )
            p2 = ps.tile([12, BLOC], F32)
            for kc in range(6):
                nc.tensor.matmul(
                    p2, lhsT=w2t[:, kc, :], rhs=r1[:, kc, :],
                    start=(kc == 0), stop=(kc == 5),
                )
            r2 = sb.tile([13, BLOC], F32R)
            # ones row lives at partition 12 (not 32-aligned): fill the whole
            # tile with 1.0 first, then overwrite rows 0..11 via ACT
            nc.vector.memset(r2.bitcast(F32), 1.0)
            nc.scalar.activation(r2[0:12], p2, AF.Relu, bias=c2)
            p3 = ps.tile([BLOC, C], F32)
            nc.tensor.matmul(p3, lhsT=r2, rhs=w3t, start=True, stop=True)
            ot = sb.tile([BLOC, C], F32)
            nc.vector.tensor_copy(ot, p3)
            nc.sync.dma_start(out=y3p, in_=ot)
    nc.compile()
    return nc


# ---------------------------------------------------------------- host glue
def _prep_stage1_inputs(x, conv_w, conv_b):
    """Build per-core stage-1 in_maps. conv_w: (4H, 1+H, 3) with out-channel
    order [i(64), f(64), o(64), g(64)] and ci order [x, h0..h63]."""
    import ml_dtypes
    bf = ml_dtypes.bfloat16
    f32 = np.float32
    w = np.asarray(conv_w, f32)
    b = np.asarray(conv_b, f32)
    oc_fi = np.concatenate([np.arange(64, 128), np.arange(0, 64)])    # [f|i]
    oc_og = np.concatenate([np.arange(128, 192), np.arange(192, 256)])  # [o|g]

    def wa(ocm):
        out = np.zeros((128, 128), f32)
        out[0:64] = w[ocm, 1:65, 1].T       # h tap 1
        out[64:128] = w[ocm, 1:65, 2].T     # h tap 2
        return out.astype(bf)

    def wb(ocm):
        out = np.zeros((68, 128), f32)
        out[0:64] = w[ocm, 1:65, 0].T       # h tap 0
        out[64] = w[ocm, 0, 0]              # x taps
        out[65] = w[ocm, 0, 1]
        out[66] = w[ocm, 0, 2]
        out[67] = b[ocm]                    # bias row
        return out.astype(bf)

    wafi, waog = wa(oc_fi), wa(oc_og)
    wbfi, wbog = wb(oc_fi), wb(oc_og)
    maps = []
    for c in range(NC):
        xt = x[c * BLOC : (c + 1) * BLOC].transpose(1, 0, 2)  # (S, BLOC, L)
        x4 = np.zeros((S, 4, BLOC, L), f32)
        x4[:, 0, :, 1:] = xt[:, :, : L - 1]   # x[c-1]
        x4[:, 1] = xt                         # x[c]
        x4[:, 2, :, : L - 1] = xt[:, :, 1:]   # x[c+1]
        x4[:, 3] = 1.0                        # bias ones row
        maps.append({
            "xp4": x4.astype(bf), "wafi": wafi, "waog": waog,
            "wbfi": wbfi, "wbog": wbog,
        })
    return maps


last_hw_ns = None
last_stage_ns = None
last_trace = None


def _run(nc, maps, label):
    trace = bool(int(os.environ.get("BASSK_TRACE", "0")))
    res = run_bass_kernel_spmd(nc, maps, core_ids=CORE_IDS, trace=trace)
    if trace:
        global last_stage_ns, last_trace
        if last_stage_ns is None:
            last_stage_ns = {}
        if last_trace is None:
            last_trace = {}
        last_stage_ns[label] = res.exec_time_ns
        it = getattr(res, "instructions_and_trace", None)
        last_trace[label] = it[1] if it else None
    return res


def kernel(**inputs):
    global last_hw_ns, last_stage_ns
    last_stage_ns = None
    f32 = np.float32
    x = np.asarray(inputs["x"], f32)
    conv_w = np.asarray(inputs["conv_w"], f32)
    conv_b = np.asarray(inputs["conv_b"], f32)
    w1 = np.asarray(inputs["w1"], f32)
    b1 = np.asarray(inputs["b1"], f32)
    g1, be1 = np.asarray(inputs["g1"], f32), np.asarray(inputs["be1"], f32)
    m1, v1 = np.asarray(inputs["m1"], f32), np.asarray(inputs["v1"], f32)
    w2 = np.asarray(inputs["w2"], f32)
    b2 = np.asarray(inputs["b2"], f32)
    g2, be2 = np.asarray(inputs["g2"], f32), np.asarray(inputs["be2"], f32)
    m2, v2 = np.asarray(inputs["m2"], f32), np.asarray(inputs["v2"], f32)
    w3 = np.asarray(inputs["w3"], f32)
    b3 = np.asarray(inputs["b3"], f32)

    # ---- stage 1: ConvLSTM (batch-parallel)
    nc1 = _build_stage1()
    maps1 = _prep_stage1_inputs(x, conv_w, conv_b)
    res1 = _run(nc1, maps1, "stage1")
    import ml_dtypes
    bf = ml_dtypes.bfloat16
    hs_all = np.stack([res1.results[c]["hs"] for c in range(NC)])  # (8,S,H,32,L) bf16

    # ---- reshard: (8,S,H,32,L) -> flatT (S*H*L, 256), feature-major, bf16
    flatT = np.ascontiguousarray(
        hs_all.transpose(1, 2, 4, 0, 3)
    ).reshape(KTOT, B)

    # ---- stage 2: big GEMM, contract-dim sharded
    s1 = g1 / np.sqrt(v1 + EPS)
    c1 = b1 * s1 + (be1 - m1 * s1)
    w1sT = np.ascontiguousarray((w1 * s1[:, None]).T).astype(bf)    # (KTOT, 768)
    nc2 = _build_stage2()
    maps2 = []
    for c in range(NC):
        sl = slice(c * KSH, (c + 1) * KSH)
        # partition-major layouts: [128, KCH*768] / [128, KCH*B] so each DMA
        # batch reads contiguous bytes per partition
        wpp = np.ascontiguousarray(
            w1sT[sl].reshape(KCH, 128, 768).transpose(1, 0, 2)
        ).reshape(128, KCH * 768)
        ftp = np.ascontiguousarray(
            flatT[sl].reshape(KCH, 128, B).transpose(1, 0, 2)
        ).reshape(128, KCH * B)
        maps2.append({"w1p": wpp, "ft": ftp})
    res2 = _run(nc2, maps2, "stage2")
    y1 = np.sum([res2.results[c]["y1p"] for c in range(NC)], axis=0,
                dtype=np.float64).astype(f32)                       # (768, 256)

    # ---- stage 3: epilogue (batch-parallel)
    s2 = g2 / np.sqrt(v2 + EPS)
    c2 = b2 * s2 + (be2 - m2 * s2)
    c1t = np.ascontiguousarray(c1.reshape(6, 128).T, f32)           # (128, 6)
    w2p = np.ascontiguousarray(
        (w2 * s2[:, None]).T.reshape(6, 128, 12), f32
    )
    w3e = np.concatenate([w3.T, b3[None, :]], axis=0).astype(f32)   # (13, 10)
    nc3 = _build_stage3()
    maps3 = []
    for c in range(NC):
        ysl = np.ascontiguousarray(
            y1[:, c * BLOC : (c + 1) * BLOC]
        ).reshape(6, 128, BLOC)
        maps3.append({
            "y1s": ysl, "c1t": c1t, "w2p": w2p,
            "c2t": c2.reshape(12, 1).astype(f32), "w3e": w3e,
        })
    res3 = _run(nc3, maps3, "stage3")
    y3 = np.concatenate([res3.results[c]["y3p"] for c in range(NC)], axis=0)
    if last_stage_ns and all(v is not None for v in last_stage_ns.values()):
        last_hw_ns = sum(last_stage_ns.values())
    return np.ascontiguousarray(y3, f32)



# revision 39
# speedup vs baseline: 1.1320x; 1.1320x over previous
"""Trainium2 Bass kernel for nn_CropConvLSTM.

Model: ConvLSTM (Conv1d(1+H -> 4H, k=3, pad=1), S=12 steps) over x (B=256,
S=12, L=128), then head Linear(98304->768)+BN+ReLU, Linear(768->12)+BN+ReLU,
Linear(12->10).

Distribution over 8 NeuronCores, three launches:
  Stage 1: ConvLSTM, data-parallel over batch (32 samples/core). Conv done as
    3 shifted fp32r matmuls (K=66: 64 h-channels + x-row + ones-row for the
    conv bias) accumulating in PSUM; gates on ACT/DVE/GPSIMD with all tensors
    at legal partition bases (tanh computed as 2*sigmoid(2x)-1 so a single
    per-partition-scaled sigmoid covers the [o;g] psum tile).
  Stage 2: y1 = flat @ (w1*bn1_scale).T, sharded over the 98304 contract dim
    (12288 features/core); each core emits a partial (768, 256), host reduces.
  Stage 3: bias+ReLU, Linear2+BN+ReLU, Linear3 (+b3 via ones-row trick),
    data-parallel over batch again.

BN (eval mode) is folded into the weights/biases on the host.
"""
import os
import sys

sys.path.insert(0, "/opt/trn_rl_repo")

from functools import lru_cache

import numpy as np

import concourse.bass as bass
import concourse.tile as tile
from concourse import bacc, mybir
from concourse.bass_utils import run_bass_kernel_spmd

F32 = mybir.dt.float32
F32R = mybir.dt.float32r
BF16 = mybir.dt.bfloat16
F16 = mybir.dt.float16
AF = mybir.ActivationFunctionType

B, S, L, H, C = 256, 12, 128, 64, 10
NC = 8
BLOC = B // NC            # 32 samples per core in stages 1/3
KTOT = S * H * L          # 98304
KSH = KTOT // NC          # 12288 contract features per core in stage 2
KCH = KSH // 128          # 96 k-chunks per core
EPS = 1e-5
CORE_IDS = list(range(NC))


# ---------------------------------------------------------------- stage 1
@lru_cache(maxsize=1)
def _build_stage1():
    """ConvLSTM step via two tap-packed matmul passes per gate pair.

    For output position l the conv contracts h at l-1, l, l+1 plus x at the
    same three offsets plus the bias.  Pass A (K=128) contracts h-taps 1 and 2
    from combA, whose partition rows hold h twice at different column shifts:
      combA[0:64,  :, c] = h[c]      (tap 1 at slice offset 0)
      combA[64:128,:, c] = h[c+1]    (tap 2; col L-1 stays 0)
    Pass B (K=68) contracts h-tap 0, the three x taps, and the bias from
    combB:
      combB[0:64, :, c] = h[c-1]     (col 0 stays 0)
      combB[64:67,:, c] = x[c-1], x[c], x[c+1]   (host-preshifted, per step)
      combB[67,  :, c] = 1.0                     (bias row)
    Gate math per group of 8 samples on [c; tanh(g)]-packed tiles so the DVE
    ops run 128 partitions wide; tanh(c) is computed directly on ACT (tanh
    and sigmoid share the act table, so no table reloads).  Groups are
    software-pipelined with a one-group lag so no engine queue blocks on the
    recurrence chain of the group just computed.
    """
    nc = bacc.Bacc("TRN2", target_bir_lowering=False, debug=False, num_devices=NC)
    xp4 = nc.dram_tensor("xp4", [S, 4, BLOC, L], BF16, kind="ExternalInput").ap()
    wafi = nc.dram_tensor("wafi", [128, 128], BF16, kind="ExternalInput").ap()
    waog = nc.dram_tensor("waog", [128, 128], BF16, kind="ExternalInput").ap()
    wbfi = nc.dram_tensor("wbfi", [68, 128], BF16, kind="ExternalInput").ap()
    wbog = nc.dram_tensor("wbog", [68, 128], BF16, kind="ExternalInput").ap()
    hs = nc.dram_tensor("hs", [S, H, BLOC, L], BF16, kind="ExternalOutput").ap()

    G = 4                 # independent sample groups (recurrence pipelines)
    GB = BLOC // G        # 8 samples per group

    with tile.TileContext(nc) as tc:
        with (
            tc.tile_pool(name="persist", bufs=1) as pp,
            tc.tile_pool(name="work", bufs=4) as wk,
            tc.tile_pool(name="psfi", bufs=2, space="PSUM") as ps_fi,
            tc.tile_pool(name="psog", bufs=2, space="PSUM") as ps_og,
        ):
            combA = pp.tile([128, BLOC, L], BF16)
            combB = pp.tile([68, BLOC, L + 2], BF16)
            wA_fi = pp.tile([128, 128], BF16)
            wA_og = pp.tile([128, 128], BF16)
            wB_fi = pp.tile([68, 128], BF16)
            wB_og = pp.tile([68, 128], BF16)
            scv = pp.tile([128, 1], F32)                # act scale [1;2]
            # per-group persistent cell-state tiles
            ct = [pp.tile([64, GB, L], BF16, name=f"ct{i}") for i in range(G)]

            # h and c start at zero, and step 0 skips every term that reads
            # them, so only the two permanent zero-pad columns need memsets:
            # combA col L-1 of the tap-2 rows and combB col 0 of the h rows
            nc.vector.memset(combA[64:128, :, L - 1 : L], 0.0)
            nc.vector.memset(combB[0:64, :, 0:1], 0.0)
            nc.vector.memset(scv[0:64], 1.0)
            nc.vector.memset(scv[64:128], 2.0)
            nc.sync.dma_start(out=wA_fi, in_=wafi)
            nc.sync.dma_start(out=wA_og, in_=waog)
            nc.sync.dma_start(out=wB_fi, in_=wbfi)
            nc.sync.dma_start(out=wB_og, in_=wbog)

            pend = []

            def emit_tail(s, g, g0, ctg, so):
                # tanh(c) then h = sig(o)*tanh(c); h lands in combA rows 0:64
                # (contiguous); the tap-2 shifted copy and the tap-0 copy into
                # combB ride SBUF->SBUF DMAs, spread across the SP and Pool
                # rings so neither ring backlogs the recurrence chain.
                tcf = wk.tile([64, GB, L], BF16, name="tcf")
                nc.scalar.activation(tcf, ctg, AF.Tanh)
                nc.vector.tensor_mul(combA[0:64, g0 : g0 + GB, :], so[0:64], tcf)
                nc.sync.dma_start(
                    out=combA[64:128, g0 : g0 + GB, 0 : L - 1],
                    in_=combA[0:64, g0 : g0 + GB, 1:L],
                )
                nc.gpsimd.dma_start(
                    out=combB[0:64, g0 : g0 + GB, 1 : L + 1],
                    in_=combA[0:64, g0 : g0 + GB, :],
                )
                nc.gpsimd.dma_start(
                    out=hs[s, :, g0 : g0 + GB, :],
                    in_=combA[0:64, g0 : g0 + GB, :],
                )

            for g in range(G):
                # step-0 x taps (later steps prefetch a full step ahead)
                nc.sync.dma_start(
                    out=combB[64:68, g * GB : (g + 1) * GB, 0:L],
                    in_=xp4[0, :, g * GB : (g + 1) * GB, :],
                )
            for s in range(S):
                for g in range(G):
                    g0 = g * GB
                    pfi = ps_fi.tile([128, GB, L], F32, name="pfi")
                    pog = ps_og.tile([128, GB, L], F32, name="pog")
                    if s > 0:
                        # h == 0 at step 0: pass A contributes nothing then
                        for wt, ps in ((wA_fi, pfi), (wA_og, pog)):
                            for c4 in range(GB // 4):
                                c0 = c4 * 4
                                nc.tensor.matmul(
                                    ps[:, c0 : c0 + 4, :], lhsT=wt,
                                    rhs=combA[:, g0 + c0 : g0 + c0 + 4, :],
                                    start=True, stop=False,
                                )
                    for wt, ps in ((wB_fi, pfi), (wB_og, pog)):
                        for c4 in range(GB // 4):
                            c0 = c4 * 4
                            if s == 0:
                                # contract only the x/ones rows (K=4)
                                nc.tensor.matmul(
                                    ps[:, c0 : c0 + 4, :], lhsT=wt[64:68],
                                    rhs=combB[64:68, g0 + c0 : g0 + c0 + 4, 0:L],
                                    start=True, stop=True,
                                )
                            else:
                                nc.tensor.matmul(
                                    ps[:, c0 : c0 + 4, :], lhsT=wt,
                                    rhs=combB[0:68, g0 + c0 : g0 + c0 + 4, 0:L],
                                    start=False, stop=True,
                                )
                    if s + 1 < S:
                        # prefetch next step's x taps now that pass B is done
                        # reading this step's rows (ACT ring; tiny transfer)
                        nc.sync.dma_start(
                            out=combB[64:68, g0 : g0 + GB, 0:L],
                            in_=xp4[s + 1, :, g0 : g0 + GB, :],
                        )
                    sg = wk.tile([128, GB, L], BF16, name="sg")
                    # [sig(o); sig(2g)] in fp16: enough mantissa that the
                    # 2x-1 tanh unfold stays accurate, and 2-byte operands
                    # keep the downstream DVE ops in the fast 2x mode
                    so = wk.tile([128, GB, L], F16, name="so")
                    nc.scalar.activation(sg, pfi, AF.Sigmoid)
                    nc.scalar.activation(so, pog, AF.Sigmoid, scale=scv)
                    ctg = ct[g]
                    # tanh(g) = 2*sig(2g)-1, staged at partitions 64:127 so
                    # the t2 multiply sees both inputs on the same partitions
                    tg = wk.tile([128, GB, L], BF16, name="tg")
                    nc.vector.tensor_scalar(
                        out=tg[64:128], in0=so[64:128], scalar1=2.0,
                        scalar2=-1.0, op0=mybir.AluOpType.mult,
                        op1=mybir.AluOpType.add,
                    )
                    if s == 0:
                        # c0 = sig(i)*tanh(g) directly (f-term is zero)
                        nc.vector.tensor_mul(ctg, sg[64:128], tg[64:128])
                    else:
                        t1 = wk.tile([64, GB, L], BF16, name="t1")
                        nc.vector.tensor_mul(t1, sg[0:64], ctg)
                        t2 = wk.tile([64, GB, L], BF16, name="t2")
                        nc.vector.tensor_mul(t2, sg[64:128], tg[64:128])
                        nc.vector.tensor_add(ctg, t1, t2)
                    pend.append((s, g, g0, ctg, so))
                    if len(pend) > 1:
                        emit_tail(*pend.pop(0))
            while pend:
                emit_tail(*pend.pop(0))
    nc.compile()
    return nc


# ---------------------------------------------------------------- stage 2
@lru_cache(maxsize=1)
def _build_stage2():
    """Contract-sharded GEMM.  Weights and activations are pre-transposed on
    the host to partition-major layout so every DMA reads fully contiguous
    bytes per partition; the weight stream (3x the activation bytes) is
    split across two hardware DMA queues (SP + ACT)."""
    nc = bacc.Bacc("TRN2", target_bir_lowering=False, debug=False, num_devices=NC)
    w1p = nc.dram_tensor("w1p", [128, KCH * 768], BF16, kind="ExternalInput").ap()
    ft = nc.dram_tensor("ft", [128, KCH * B], BF16, kind="ExternalInput").ap()
    y1p = nc.dram_tensor("y1p", [B, 768], F32, kind="ExternalOutput").ap()

    KB = 8                       # k-chunks per DMA batch
    NB = KCH // KB               # 12 batches

    with tile.TileContext(nc) as tc:
        with (
            tc.tile_pool(name="wp", bufs=3) as wp,
            tc.tile_pool(name="rp", bufs=3) as rp,
            tc.tile_pool(name="op", bufs=2) as op,
            tc.tile_pool(name="ps", bufs=1, space="PSUM") as ps,
        ):
            # activations stationary, weights moving: 384 matmuls (N=512/256)
            # instead of 576 of N=256 -> fewer LDWEIGHTS serializations; the
            # output lands transposed (psum rows = samples).  Accumulators are
            # padded to 2 full PSUM banks and N is split 512+256 so no matmul
            # output region crosses a bank boundary.
            acc = [ps.tile([128, 1024], F32, name=f"acc{bb}") for bb in range(2)]
            WB = KB * 768
            RB = KB * B
            for kb in range(NB):
                wt = wp.tile([128, WB], BF16, name="wt")
                rt = rp.tile([128, RB], BF16, name="rt")
                nc.sync.dma_start(
                    out=wt[:, 0 : WB // 2],
                    in_=w1p[:, kb * WB : kb * WB + WB // 2],
                )
                nc.scalar.dma_start(
                    out=wt[:, WB // 2 : WB],
                    in_=w1p[:, kb * WB + WB // 2 : (kb + 1) * WB],
                )
                nc.gpsimd.dma_start(
                    out=rt, in_=ft[:, kb * RB : (kb + 1) * RB],
                )
                for kc in range(KB):
                    for bb in range(2):
                        lhsT = rt[:, kc * B + bb * 128 : kc * B + (bb + 1) * 128]
                        for n0, n1 in ((0, 512), (512, 768)):
                            nc.tensor.matmul(
                                acc[bb][:, n0:n1],
                                lhsT=lhsT,
                                rhs=wt[:, kc * 768 + n0 : kc * 768 + n1],
                                start=(kb == 0 and kc == 0),
                                stop=(kb == NB - 1 and kc == KB - 1),
                            )
            for bb in range(2):
                ot = op.tile([128, 768], F32, name="ot")
                nc.vector.tensor_copy(ot, acc[bb][:, 0:768])
                nc.sync.dma_start(out=y1p[bb * 128 : (bb + 1) * 128], in_=ot)
    nc.compile()
    return nc


# ---------------------------------------------------------------- stage 3
@lru_cache(maxsize=1)
def _build_stage3():
    nc = bacc.Bacc("TRN2", target_bir_lowering=False, debug=False, num_devices=NC)
    y1s = nc.dram_tensor("y1s", [6, 128, BLOC], F32R, kind="ExternalInput").ap()
    c1t = nc.dram_tensor("c1t", [128, 6], F32, kind="ExternalInput").ap()
    w2p = nc.dram_tensor("w2p", [6, 128, 12], F32R, kind="ExternalInput").ap()
    c2t = nc.dram_tensor("c2t", [12, 1], F32, kind="ExternalInput").ap()
    w3e = nc.dram_tensor("w3e", [13, 10], F32R, kind="ExternalInput").ap()
    y3p = nc.dram_tensor("y3p", [BLOC, C], F32, kind="ExternalOutput").ap()

    with tile.TileContext(nc) as tc:
        with (
            tc.tile_pool(name="sb", bufs=1) as sb,
            tc.tile_pool(name="ps", bufs=1, space="PSUM") as ps,
        ):
            # warm the ACT table (relu set) while the input DMAs run
            warm = sb.tile([1, 1], F32)
            nc.vector.memset(warm, 0.0)
            nc.scalar.activation(warm, warm, AF.Relu)
            yt = sb.tile([128, 6, BLOC], F32R)
            c1 = sb.tile([128, 6], F32)
            w2t = sb.tile([128, 6, 12], F32R)
            c2 = sb.tile([12, 1], F32)
            w3t = sb.tile([13, 10], F32R)
            nc.sync.dma_start(out=yt, in_=y1s.rearrange("k p b -> p k b"))
            nc.gpsimd.dma_start(out=c1, in_=c1t)
            nc.scalar.dma_start(out=w2t, in_=w2p.rearrange("k p m -> p k m"))
            nc.gpsimd.dma_start(out=c2, in_=c2t)
            nc.scalar.dma_start(out=w3t, in_=w3e)

            r1 = sb.tile([128, 6, BLOC], F32R)
            for kc in range(6):
                nc.scalar.activation(
                    r1[:, kc, :], yt[:, kc, :], AF.Relu, bias=c1[:, kc : kc + 1]
                )
            p2 = ps.tile([12, BLOC], F32)
            for kc in range(6):
                nc.tensor.matmul(
                    p2, lhsT=w2t[:, kc, :], rhs=r1[:, kc, :],
                    start=(kc == 0), stop=(kc == 5),
                )
            r2 = sb.tile([13, BLOC], F32R)
            # ones row lives at partition 12 (not 32-aligned): fill the whole
            # tile with 1.0 first, then overwrite rows 0..11 via ACT
            nc.vector.memset(r2.bitcast(F32), 1.0)
            nc.scalar.activation(r2[0:12], p2, AF.Relu, bias=c2)
            p3 = ps.tile([BLOC, C], F32)
            nc.tensor.matmul(p3, lhsT=r2, rhs=w3t, start=True, stop=True)
            ot = sb.tile([BLOC, C], F32)
            nc.vector.tensor_copy(ot, p3)
            nc.sync.dma_start(out=y3p, in_=ot)
    nc.compile()
    return nc


# ---------------------------------------------------------------- host glue
def _prep_stage1_inputs(x, conv_w, conv_b):
    """Build per-core stage-1 in_maps. conv_w: (4H, 1+H, 3) with out-channel
    order [i(64), f(64), o(64), g(64)] and ci order [x, h0..h63]."""
    import ml_dtypes
    bf = ml_dtypes.bfloat16
    f32 = np.float32
    w = np.asarray(conv_w, f32)
    b = np.asarray(conv_b, f32)
    oc_fi = np.concatenate([np.arange(64, 128), np.arange(0, 64)])    # [f|i]
    oc_og = np.concatenate([np.arange(128, 192), np.arange(192, 256)])  # [o|g]

    def wa(ocm):
        out = np.zeros((128, 128), f32)
        out[0:64] = w[ocm, 1:65, 1].T       # h tap 1
        out[64:128] = w[ocm, 1:65, 2].T     # h tap 2
        return out.astype(bf)

    def wb(ocm):
        out = np.zeros((68, 128), f32)
        out[0:64] = w[ocm, 1:65, 0].T       # h tap 0
        out[64] = w[ocm, 0, 0]              # x taps
        out[65] = w[ocm, 0, 1]
        out[66] = w[ocm, 0, 2]
        out[67] = b[ocm]                    # bias row
        return out.astype(bf)

    wafi, waog = wa(oc_fi), wa(oc_og)
    wbfi, wbog = wb(oc_fi), wb(oc_og)
    maps = []
    for c in range(NC):
        xt = x[c * BLOC : (c + 1) * BLOC].transpose(1, 0, 2)  # (S, BLOC, L)
        x4 = np.zeros((S, 4, BLOC, L), f32)
        x4[:, 0, :, 1:] = xt[:, :, : L - 1]   # x[c-1]
        x4[:, 1] = xt                         # x[c]
        x4[:, 2, :, : L - 1] = xt[:, :, 1:]   # x[c+1]
        x4[:, 3] = 1.0                        # bias ones row
        maps.append({
            "xp4": x4.astype(bf), "wafi": wafi, "waog": waog,
            "wbfi": wbfi, "wbog": wbog,
        })
    return maps


last_hw_ns = None
last_stage_ns = None
last_trace = None


def _run(nc, maps, label):
    trace = bool(int(os.environ.get("BASSK_TRACE", "0")))
    res = run_bass_kernel_spmd(nc, maps, core_ids=CORE_IDS, trace=trace)
    if trace:
        global last_stage_ns, last_trace
        if last_stage_ns is None:
            last_stage_ns = {}
        if last_trace is None:
            last_trace = {}
        last_stage_ns[label] = res.exec_time_ns
        it = getattr(res, "instructions_and_trace", None)
        last_trace[label] = it[1] if it else None
    return res


def kernel(**inputs):
    global last_hw_ns, last_stage_ns
    last_stage_ns = None
    f32 = np.float32
    x = np.asarray(inputs["x"], f32)
    conv_w = np.asarray(inputs["conv_w"], f32)
    conv_b = np.asarray(inputs["conv_b"], f32)
    w1 = np.asarray(inputs["w1"], f32)
    b1 = np.asarray(inputs["b1"], f32)
    g1, be1 = np.asarray(inputs["g1"], f32), np.asarray(inputs["be1"], f32)
    m1, v1 = np.asarray(inputs["m1"], f32), np.asarray(inputs["v1"], f32)
    w2 = np.asarray(inputs["w2"], f32)
    b2 = np.asarray(inputs["b2"], f32)
    g2, be2 = np.asarray(inputs["g2"], f32), np.asarray(inputs["be2"], f32)
    m2, v2 = np.asarray(inputs["m2"], f32), np.asarray(inputs["v2"], f32)
    w3 = np.asarray(inputs["w3"], f32)
    b3 = np.asarray(inputs["b3"], f32)

    # ---- stage 1: ConvLSTM (batch-parallel)
    nc1 = _build_stage1()
    maps1 = _prep_stage1_inputs(x, conv_w, conv_b)
    res1 = _run(nc1, maps1, "stage1")
    import ml_dtypes
    bf = ml_dtypes.bfloat16
    hs_all = np.stack([res1.results[c]["hs"] for c in range(NC)])  # (8,S,H,32,L) bf16

    # ---- reshard: (8,S,H,32,L) -> flatT (S*H*L, 256), feature-major, bf16
    flatT = np.ascontiguousarray(
        hs_all.transpose(1, 2, 4, 0, 3)
    ).reshape(KTOT, B)

    # ---- stage 2: big GEMM, contract-dim sharded
    s1 = g1 / np.sqrt(v1 + EPS)
    c1 = b1 * s1 + (be1 - m1 * s1)
    w1sT = np.ascontiguousarray((w1 * s1[:, None]).T).astype(bf)    # (KTOT, 768)
    nc2 = _build_stage2()
    maps2 = []
    for c in range(NC):
        sl = slice(c * KSH, (c + 1) * KSH)
        # partition-major layouts: [128, KCH*768] / [128, KCH*B] so each DMA
        # batch reads contiguous bytes per partition
        wpp = np.ascontiguousarray(
            w1sT[sl].reshape(KCH, 128, 768).transpose(1, 0, 2)
        ).reshape(128, KCH * 768)
        ftp = np.ascontiguousarray(
            flatT[sl].reshape(KCH, 128, B).transpose(1, 0, 2)
        ).reshape(128, KCH * B)
        maps2.append({"w1p": wpp, "ft": ftp})
    res2 = _run(nc2, maps2, "stage2")
    y1 = np.sum([res2.results[c]["y1p"] for c in range(NC)], axis=0,
                dtype=np.float64).astype(f32).T                     # (768, 256)

    # ---- stage 3: epilogue (batch-parallel)
    s2 = g2 / np.sqrt(v2 + EPS)
    c2 = b2 * s2 + (be2 - m2 * s2)
    c1t = np.ascontiguousarray(c1.reshape(6, 128).T, f32)           # (128, 6)
    w2p = np.ascontiguousarray(
        (w2 * s2[:, None]).T.reshape(6, 128, 12), f32
    )
    w3e = np.concatenate([w3.T, b3[None, :]], axis=0).astype(f32)   # (13, 10)
    nc3 = _build_stage3()
    maps3 = []
    for c in range(NC):
        ysl = np.ascontiguousarray(
            y1[:, c * BLOC : (c + 1) * BLOC]
        ).reshape(6, 128, BLOC)
        maps3.append({
            "y1s": ysl, "c1t": c1t, "w2p": w2p,
            "c2t": c2.reshape(12, 1).astype(f32), "w3e": w3e,
        })
    res3 = _run(nc3, maps3, "stage3")
    y3 = np.concatenate([res3.results[c]["y3p"] for c in range(NC)], axis=0)
    if last_stage_ns and all(v is not None for v in last_stage_ns.values()):
        last_hw_ns = sum(last_stage_ns.values())
    return np.ascontiguousarray(y3, f32)

